# revision 11
# baseline (speedup 1.0000x reference)
"""Trainium2 Bass kernel for a dense transformer block (attention + FFN).

Problem shapes: x [2, 2048, 1024], H=16 heads of 64, FFN 4096, fp32 I/O.

Sharding: token-parallel over 8 cores. Core c handles batch b = c // 4 and
query rows qoff = (c % 4) * 512. Each core recomputes K/V for its whole
batch element (2048 tokens), so no cross-core collectives are needed;
outputs are disjoint row-slices concatenated on the host.

Per-core dataflow (feature-on-partition "T" layouts; out = lhsT.T @ rhs):
  XT  = x^T via PE transposes (bf16)                    [1024, 2048]
  QT/KT/VT = W^T @ XT (bf16 matmuls)                    [*, tokens]
  V'  = transpose(VT) per 128-key tile, with a ones column appended per head
  ST  = exp((KT_h^T @ QT_h) / 8)  (no max-subtraction: |scores| < ~3)
  OT' = V'^T @ ST  -> rows 0..63 = unnormalized out^T, row 64 = sumexp
  transpose OT' -> per-q-tile [128, 65]; divide by col 64; residual + LN1
  FFN: H1T = gelu(W1^T @ resT + b1) in float32r; out2T = W2^T @ H1T + b2
  transpose back, residual + LN2 -> out rows [512, 1024] f32
"""
import sys
sys.path.insert(0, "/opt/trn_rl_repo")

import numpy as np
import ml_dtypes

import concourse.bass as bass
import concourse.mybir as mybir
import concourse.tile as tile
from concourse import bacc
from concourse.bass_utils import run_bass_kernel_spmd

F32 = mybir.dt.float32
F32R = mybir.dt.float32r
BF16 = mybir.dt.bfloat16
AF = mybir.ActivationFunctionType
ALU = mybir.AluOpType

B, S, D = 2, 2048, 1024
H, HD = 16, 64
DFF = 4096
TQ = 512            # query tokens per core
NCORES = 8
EPS = 1e-5

USE_GELU = True     # CoreSim has no Gelu LUT; test.py flips this for sim runs


def _col_tile_ap(dram_vec, n_tiles):
    """[n*128] dram vector -> [128, n] AP (tile t in column t)."""
    return bass.AP(tensor=dram_vec[:].tensor, offset=0,
                   ap=[[1, 128], [128, n_tiles]])


def _rep_ap(dram_vec, n):
    """[n] dram vector -> [128, n] AP broadcast across partitions."""
    return bass.AP(tensor=dram_vec[:].tensor, offset=0, ap=[[0, 128], [1, n]])


def build(repeat=1):
    nc = bacc.Bacc()

    # ---------------- I/O ----------------
    xT16 = nc.dram_tensor("xT16", [D, S], BF16, kind="ExternalInput")
    xqT16 = nc.dram_tensor("xqT16", [D, TQ], BF16, kind="ExternalInput")
    xqf = nc.dram_tensor("xqf", [TQ, D], F32, kind="ExternalInput")
    wq16 = nc.dram_tensor("wq16", [D, D], BF16, kind="ExternalInput")
    wk16 = nc.dram_tensor("wk16", [D, D], BF16, kind="ExternalInput")
    wv16 = nc.dram_tensor("wv16", [D, D], BF16, kind="ExternalInput")
    w1 = nc.dram_tensor("w1", [D, DFF], F32R, kind="ExternalInput")
    w2 = nc.dram_tensor("w2", [DFF, D], F32R, kind="ExternalInput")
    bq = nc.dram_tensor("bq", [D], F32, kind="ExternalInput")
    bk = nc.dram_tensor("bk", [D], F32, kind="ExternalInput")
    bv = nc.dram_tensor("bv", [D], F32, kind="ExternalInput")
    b1d = nc.dram_tensor("b1d", [DFF], F32, kind="ExternalInput")
    b2d = nc.dram_tensor("b2d", [D], F32, kind="ExternalInput")
    g1d = nc.dram_tensor("g1d", [D], F32, kind="ExternalInput")
    be1d = nc.dram_tensor("be1d", [D], F32, kind="ExternalInput")
    g2d = nc.dram_tensor("g2d", [D], F32, kind="ExternalInput")
    be2d = nc.dram_tensor("be2d", [D], F32, kind="ExternalInput")
    id16d = nc.dram_tensor("id16d", [128, 128], BF16, kind="ExternalInput")
    idr32d = nc.dram_tensor("idr32d", [128, 128], F32R, kind="ExternalInput")
    out = nc.dram_tensor("out", [TQ, D], F32, kind="ExternalOutput")

    DT = D // 128       # 8 feature tiles
    ST_ = S // 128      # 16 key tiles
    QT_ = TQ // 128     # 4 query-row tiles
    NP = H // 2         # 8 head pairs

    with tile.TileContext(nc) as tc:
      for _rep in range(repeat):
        with tc.tile_pool(name="consts", bufs=1) as consts, \
             tc.tile_pool(name="persist", bufs=1) as persist:
            # ---- constants / biases ----
            id16 = consts.tile([128, 128], BF16)
            nc.sync.dma_start(out=id16, in_=id16d[:, :])
            idr = consts.tile([128, 128], F32R)
            nc.sync.dma_start(out=idr, in_=idr32d[:, :])
            eps_t = consts.tile([128, 1], F32)
            nc.vector.memset(eps_t, EPS)
            bq_t = consts.tile([128, DT], F32)
            nc.sync.dma_start(out=bq_t, in_=_col_tile_ap(bq, DT))
            bk_t = consts.tile([128, NP], F32)
            nc.sync.dma_start(out=bk_t, in_=_col_tile_ap(bk, NP))
            bv_t = consts.tile([128, NP], F32)
            nc.sync.dma_start(out=bv_t, in_=_col_tile_ap(bv, NP))
            b1_t = consts.tile([128, DFF // 128], F32)
            nc.sync.dma_start(out=b1_t, in_=_col_tile_ap(b1d, DFF // 128))
            g1r = consts.tile([128, D], F32)
            nc.sync.dma_start(out=g1r, in_=_rep_ap(g1d, D))
            be1r = consts.tile([128, D], F32)
            nc.sync.dma_start(out=be1r, in_=_rep_ap(be1d, D))
            g2r = consts.tile([128, D], F32)
            nc.sync.dma_start(out=g2r, in_=_rep_ap(g2d, D))
            be2r = consts.tile([128, D], F32)
            nc.sync.dma_start(out=be2r, in_=_rep_ap(be2d, D))

            # ---- tensors alive into the FFN phase ----
            res = persist.tile([128, QT_, D], F32R)     # LN1 output
            resT = persist.tile([128, DT, TQ], F32R)    # transposed LN1 output

            with tc.tile_pool(name="attn_sb", bufs=1) as asb, \
                 tc.tile_pool(name="attn_db", bufs=2) as adb, \
                 tc.tile_pool(name="kt_ps", bufs=2, space="PSUM") as kt_ps, \
                 tc.tile_pool(name="st_ps", bufs=3, space="PSUM") as st_ps, \
                 tc.tile_pool(name="o_ps", bufs=1, space="PSUM") as o_ps, \
                 tc.tile_pool(name="sm_ps", bufs=2, space="PSUM") as sm_ps:

                XT = asb.tile([128, DT, S], BF16)
                XTq = asb.tile([128, DT, TQ], BF16)
                QT = asb.tile([128, DT, TQ], BF16)
                O = asb.tile([128, QT_, D], F32)

                # ============ P0/P1: QT first (XTq + Wq), XT streams after
                for ft in range(DT):
                    nc.sync.dma_start(out=XTq[:, ft, :],
                                      in_=xqT16[ft * 128:(ft + 1) * 128, :])
                with tc.tile_pool(name="wq_sb", bufs=1) as wqp:
                    wq_s = wqp.tile([128, DT, D], BF16)
                    for ft in range(DT):
                        nc.sync.dma_start(out=wq_s[:, ft, :],
                                          in_=wq16[ft * 128:(ft + 1) * 128, :])
                    for qc in range(DT):
                        qp_w = st_ps.tile([128, 1024], F32, tag="st")
                        qp = qp_w[:, 0:TQ]
                        for ft in range(DT):
                            nc.tensor.matmul(
                                qp, wq_s[:, ft, qc * 128:(qc + 1) * 128],
                                XTq[:, ft, :],
                                start=(ft == 0), stop=(ft == DT - 1))
                        nc.scalar.activation(out=QT[:, qc, :], in_=qp,
                                             func=AF.Identity,
                                             bias=bq_t[:, qc:qc + 1])
                    for ft in range(DT):
                        nc.sync.dma_start(out=XT[:, ft, :],
                                          in_=xT16[ft * 128:(ft + 1) * 128, :])

                # ============ P2: head pairs ============
                for p in range(NP):
                    wk_s = adb.tile([128, DT, 128], BF16, tag="wk")
                    wv_s = adb.tile([128, DT, 128], BF16, tag="wv")
                    for ft in range(DT):
                        nc.sync.dma_start(
                            out=wk_s[:, ft, :],
                            in_=wk16[ft * 128:(ft + 1) * 128,
                                     p * 128:(p + 1) * 128])
                        nc.sync.dma_start(
                            out=wv_s[:, ft, :],
                            in_=wv16[ft * 128:(ft + 1) * 128,
                                     p * 128:(p + 1) * 128])
                    KT_p = adb.tile([128, S], BF16, tag="ktp")
                    VT_p = adb.tile([128, S], BF16, tag="vtp")
                    for ch in range(4):
                        kp = kt_ps.tile([128, 512], F32, tag="kt")
                        for ft in range(DT):
                            nc.tensor.matmul(
                                kp, wk_s[:, ft, :],
                                XT[:, ft, ch * 512:(ch + 1) * 512],
                                start=(ft == 0), stop=(ft == DT - 1))
                        nc.vector.tensor_scalar(
                            out=KT_p[:, ch * 512:(ch + 1) * 512], in0=kp,
                            scalar1=bk_t[:, p:p + 1], scalar2=None,
                            op0=ALU.add)
                    for ch in range(4):
                        vp = kt_ps.tile([128, 512], F32, tag="kt")
                        for ft in range(DT):
                            nc.tensor.matmul(
                                vp, wv_s[:, ft, :],
                                XT[:, ft, ch * 512:(ch + 1) * 512],
                                start=(ft == 0), stop=(ft == DT - 1))
                        nc.vector.tensor_scalar(
                            out=VT_p[:, ch * 512:(ch + 1) * 512], in0=vp,
                            scalar1=bv_t[:, p:p + 1], scalar2=None,
                            op0=ALU.add)

                    # V' = transpose(VT_p) + ones columns (64, 129)
                    Vp = adb.tile([128, ST_, 130], BF16, tag="vprime")
                    nc.vector.memset(Vp[:, :, 64:65], 1.0)
                    nc.vector.memset(Vp[:, :, 129:130], 1.0)
                    for kt in range(ST_):
                        vt = sm_ps.tile([128, 128], BF16, tag="sm")
                        nc.tensor.transpose(
                            vt, VT_p[:, kt * 128:(kt + 1) * 128], id16)
                        nc.vector.tensor_copy(out=Vp[:, kt, 0:64],
                                              in_=vt[:, 0:64])
                        nc.vector.tensor_copy(out=Vp[:, kt, 65:129],
                                              in_=vt[:, 64:128])

                    for i in range(2):          # heads 2p, 2p+1
                        h = 2 * p + i
                        STx = adb.tile([128, ST_, 512], BF16, tag="stexp")
                        for kt in range(ST_):
                            sp = st_ps.tile([128, 512], F32, tag="st")
                            nc.tensor.matmul(
                                sp,
                                KT_p[64 * i:64 * i + 64,
                                     kt * 128:(kt + 1) * 128],
                                QT[64 * i:64 * i + 64, p, :],
                                start=True, stop=True)
                            nc.scalar.activation(out=STx[:, kt, :], in_=sp,
                                                 func=AF.Exp, scale=0.125)
                        op = o_ps.tile([65, 512], F32, tag="o")
                        for kt in range(ST_):
                            nc.tensor.matmul(
                                op, Vp[:, kt, 65 * i:65 * i + 65],
                                STx[:, kt, :],
                                start=(kt == 0), stop=(kt == ST_ - 1))
                        ot_s = adb.tile([65, 512], BF16, tag="ots")
                        nc.vector.tensor_copy(out=ot_s, in_=op)
                        for qt in range(QT_):
                            tp2 = sm_ps.tile([128, 65], BF16, tag="sm")
                            nc.tensor.transpose(
                                tp2, ot_s[:, qt * 128:(qt + 1) * 128],
                                id16[0:65, 0:65])
                            rec = adb.tile([128, 1], F32, tag="rec")
                            nc.vector.reciprocal(out=rec, in_=tp2[:, 64:65])
                            nc.vector.tensor_scalar_mul(
                                out=O[:, qt, h * 64:(h + 1) * 64],
                                in0=tp2[:, 0:64], scalar1=rec)

                # ============ P3: residual + LN1, resT ============
                with tc.tile_pool(name="p3", bufs=1) as p3p:
                    xq_s = p3p.tile([128, QT_, D], F32)
                    for t in range(QT_):
                        nc.sync.dma_start(out=xq_s[:, t, :],
                                          in_=xqf[t * 128:(t + 1) * 128, :])
                    for qt in range(QT_):
                        nc.vector.tensor_add(out=O[:, qt, :],
                                             in0=O[:, qt, :],
                                             in1=xq_s[:, qt, :])
                        stats = p3p.tile([128, 2, 6], F32, tag="stats")
                        nc.vector.bn_stats(out=stats[:, 0, :],
                                           in_=O[:, qt, 0:512])
                        nc.vector.bn_stats(out=stats[:, 1, :],
                                           in_=O[:, qt, 512:1024])
                        mv = p3p.tile([128, 2], F32, tag="mv")
                        nc.vector.bn_aggr(out=mv, in_=stats)
                        rstd = p3p.tile([128, 1], F32, tag="rstd")
                        nc.scalar.activation(out=rstd, in_=mv[:, 1:2],
                                             func=AF.Sqrt, bias=eps_t)
                        nc.vector.reciprocal(out=rstd, in_=rstd)
                        nrm = p3p.tile([128, D], F32, tag="nrm")
                        nc.vector.tensor_scalar(
                            out=nrm, in0=O[:, qt, :], scalar1=mv[:, 0:1],
                            scalar2=rstd, op0=ALU.subtract, op1=ALU.mult)
                        nc.vector.tensor_mul(out=nrm, in0=nrm, in1=g1r)
                        nc.vector.tensor_add(out=res[:, qt, :],
                                             in0=nrm, in1=be1r)
                    for ft in range(DT):
                        for qt in range(QT_):
                            rp = sm_ps.tile([128, 128], F32R, tag="sm")
                            nc.tensor.transpose(
                                rp, res[:, qt, ft * 128:(ft + 1) * 128], idr)
                            nc.vector.tensor_copy(
                                out=resT[:, ft, qt * 128:(qt + 1) * 128],
                                in_=rp)

            # ============ P4: FFN1 (H1T = gelu(W1^T resT + b1)) ============
            with tc.tile_pool(name="ffn_sb", bufs=1) as fsb:
                H1T = fsb.tile([128, DFF // 128, TQ], F32R)
                with tc.tile_pool(name="w1_sb", bufs=2) as w1p, \
                     tc.tile_pool(name="h1_ps", bufs=8, space="PSUM") as h1ps:
                    for ch in range(8):         # chunks of 512 dff cols
                        w1t = w1p.tile([128, DT, 512], F32R, tag="w1t")
                        for ft in range(DT):
                            nc.sync.dma_start(
                                out=w1t[:, ft, :],
                                in_=w1[ft * 128:(ft + 1) * 128,
                                       ch * 512:(ch + 1) * 512])
                        for j in range(4):
                            hp = h1ps.tile([128, TQ], F32, tag="h1")
                            for ft in range(DT):
                                nc.tensor.matmul(
                                    hp, w1t[:, ft, j * 128:(j + 1) * 128],
                                    resT[:, ft, :],
                                    start=(ft == 0), stop=(ft == DT - 1))
                            jj = ch * 4 + j
                            nc.scalar.activation(
                                out=H1T[:, jj, :], in_=hp,
                                func=(AF.Gelu if USE_GELU else AF.Identity),
                                bias=b1_t[:, jj:jj + 1])

                # ============ P5: FFN2 (out2T = W2^T H1T + b2) ============
                out2T = fsb.tile([128, DT, TQ], F32)
                with tc.tile_pool(name="w2_sb", bufs=3) as w2p, \
                     tc.tile_pool(name="o2_ps", bufs=1, space="PSUM") as o2ps:
                    o2 = [o2ps.tile([128, TQ], F32, tag=f"o2_{j}", name=f"o2_{j}")
                          for j in range(DT)]
                    for dt_ in range(DFF // 128):
                        w2t = w2p.tile([128, D], F32R, tag="w2t")
                        nc.sync.dma_start(
                            out=w2t, in_=w2[dt_ * 128:(dt_ + 1) * 128, :])
                        for j in range(DT):
                            nc.tensor.matmul(
                                o2[j], w2t[:, j * 128:(j + 1) * 128],
                                H1T[:, dt_, :],
                                start=(dt_ == 0), stop=(dt_ == DFF // 128 - 1),
                                skip_group_check=True)
                    for j in range(DT):
                        nc.scalar.activation(out=out2T[:, j, :], in_=o2[j],
                                             func=AF.Identity,
                                             bias=b2_t[:, j:j + 1])

                # ============ P6: transpose back, residual + LN2, store ====
                with tc.tile_pool(name="p6", bufs=1) as p6p, \
                     tc.tile_pool(name="t2_ps", bufs=2, space="PSUM") as t2ps:
                    fin = p6p.tile([128, QT_, D], F32)
                    for j in range(DT):
                        for qt in range(QT_):
                            t2 = t2ps.tile([128, 128], F32, tag="t2")
                            nc.tensor.transpose(
                                t2, out2T[:, j, qt * 128:(qt + 1) * 128], idf)
                            nc.vector.tensor_add(
                                out=fin[:, qt, j * 128:(j + 1) * 128],
                                in0=t2,
                                in1=res[:, qt, j * 128:(j + 1) * 128].bitcast(F32))
                    for qt in range(QT_):
                        stats = p6p.tile([128, 2, 6], F32, tag="stats2")
                        nc.vector.bn_stats(out=stats[:, 0, :],
                                           in_=fin[:, qt, 0:512])
                        nc.vector.bn_stats(out=stats[:, 1, :],
                                           in_=fin[:, qt, 512:1024])
                        mv = p6p.tile([128, 2], F32, tag="mv2")
                        nc.vector.bn_aggr(out=mv, in_=stats)
                        rstd = p6p.tile([128, 1], F32, tag="rstd2")
                        nc.scalar.activation(out=rstd, in_=mv[:, 1:2],
                                             func=AF.Sqrt, bias=eps_t)
                        nc.vector.reciprocal(out=rstd, in_=rstd)
                        nc.vector.tensor_scalar(
                            out=fin[:, qt, :], in0=fin[:, qt, :],
                            scalar1=mv[:, 0:1], scalar2=rstd,
                            op0=ALU.subtract, op1=ALU.mult)
                        nc.vector.tensor_mul(out=fin[:, qt, :],
                                             in0=fin[:, qt, :], in1=g2r)
                        nc.vector.tensor_add(out=fin[:, qt, :],
                                             in0=fin[:, qt, :], in1=be2r)
                        nc.sync.dma_start(out=out[qt * 128:(qt + 1) * 128, :],
                                          in_=fin[:, qt, :])
    nc.compile()
    return nc


_NC_CACHE = {}


def _get_nc(repeat=1):
    key = (USE_GELU, repeat)
    if key not in _NC_CACHE:
        _NC_CACHE[key] = build(repeat)
    return _NC_CACHE[key]


def make_in_maps(x, Wq, bq, Wk, bk, Wv, bv, W1, b1, W2, b2, g1, be1, g2, be2):
    bf = ml_dtypes.bfloat16
    shared = {
        "wq16": np.ascontiguousarray(Wq.astype(bf)),
        "wk16": np.ascontiguousarray(Wk.astype(bf)),
        "wv16": np.ascontiguousarray(Wv.astype(bf)),
        "w1": np.ascontiguousarray(W1, dtype=np.float32),
        "w2": np.ascontiguousarray(W2, dtype=np.float32),
        "bq": np.asarray(bq, np.float32), "bk": np.asarray(bk, np.float32),
        "bv": np.asarray(bv, np.float32), "b1d": np.asarray(b1, np.float32),
        "b2d": np.asarray(b2, np.float32), "g1d": np.asarray(g1, np.float32),
        "be1d": np.asarray(be1, np.float32), "g2d": np.asarray(g2, np.float32),
        "be2d": np.asarray(be2, np.float32),
        "id16d": np.eye(128, dtype=bf),
        "idr32d": np.eye(128, dtype=np.float32),
    }
    in_maps = []
    for c in range(NCORES):
        b, chunk = divmod(c, 4)
        qoff = chunk * TQ
        xb = np.asarray(x[b], np.float32)
        m = dict(shared)
        m["xT16"] = np.ascontiguousarray(xb.T.astype(bf))
        m["xqT16"] = np.ascontiguousarray(xb[qoff:qoff + TQ].T.astype(bf))
        m["xqf"] = np.ascontiguousarray(xb[qoff:qoff + TQ])
        in_maps.append(m)
    return in_maps


def kernel(x, Wq, bq, Wk, bk, Wv, bv, W1, b1, W2, b2, g1, be1, g2, be2):
    nc = _get_nc()
    in_maps = make_in_maps(x, Wq, bq, Wk, bk, Wv, bv, W1, b1, W2, b2,
                           g1, be1, g2, be2)
    try:
        r = run_bass_kernel_spmd(nc, in_maps, list(range(NCORES)))
    except Exception:
        # transient device errors (e.g. a wedged NeuronCore) usually clear
        # on retry
        import time as _time
        _time.sleep(2)
        r = run_bass_kernel_spmd(nc, in_maps, list(range(NCORES)))
    final = np.empty((B, S, D), np.float32)
    for c in range(NCORES):
        b, chunk = divmod(c, 4)
        qoff = chunk * TQ
        final[b, qoff:qoff + TQ] = r.results[c]["out"]
    return final
